# revision 25
# baseline (speedup 1.0000x reference)
"""LSTM decoder with attention (image captioning) — Trainium2 Bass kernel.

Sharding: data-parallel over batch (64 images -> 8 cores x 8 images).
The whole per-step recurrence is collective-free; host does cheap glue
(embedding gather, weight transposes, h0/c0 init, final bias add).

Device program per core (b = 8 local images):
  pre:   enc_projT[a, (b,j,q)] = wenc @ IF.T + (wenc_b + wdec_b)
         IFW[(b,j,q), d4]      = IF @ Wc.T
  loop (t = 0..19, serial, pipelined over two b-halves):
         hprojT = wdec@h (PSUM) -> hp ; gates_T partial = W_hh@h + embproj
         per half: att = tanh(encp + hp[b]) ; e = V.att (transposed)
                   softmax (exp table) ; gates_T += sum_p w[b,p]*IFW[b,p,:]
         LSTM cell in tanh-only form: sigmoid(x) = (1+tanh(x/2))/2 with
         state rescaled (H~=2h, C~=2c, wdec/W_hh/fc pre-halved) so the
         scalar engine never loads the sigmoid table (stays on exp table).
  tail:  logits = H~.T @ (fc/2).T over all 20 steps, bf16 out.

All recurrence matmuls bf16; accumulation fp32.
"""

import os
import sys
import numpy as np

for _p in ("/opt/trn_rl_repo",):
    if _p not in sys.path and os.path.isdir(_p):
        sys.path.insert(0, _p)

import ml_dtypes  # noqa: E402

import concourse.bass as bass  # noqa: E402
import concourse.tile as tile  # noqa: E402
from concourse import bacc, mybir  # noqa: E402
from concourse.bass import ts  # noqa: E402
from concourse.bass_utils import run_bass_kernel_spmd  # noqa: E402

AF = mybir.ActivationFunctionType
ALU = mybir.AluOpType
F32 = mybir.dt.float32
BF16 = mybir.dt.bfloat16
BF = ml_dtypes.bfloat16

# problem shapes (hardcoded)
VOCAB, ENC, EMB, DEC, ATT = 10000, 2048, 512, 512, 512
B, P, S = 64, 196, 20
NCORES = 8
NB = B // NCORES          # 8 images per core
HB = NB // 2              # 4 images per half (pipelined halves)
PPAD = 256                # P padded to 2 k-tiles per image
NJ = PPAD // 128          # 2
NBJ = NB * NJ             # 16 (b,j) k-tiles
NE = ENC // 128           # 16
NA = ATT // 128           # 4
ND = DEC // 128           # 4
NG = (4 * DEC) // 128     # 16 gate tiles
D4 = 4 * DEC              # 2048
NVC = 20                  # vocab chunks
VC = VOCAB // NVC         # 500
VH = VOCAB // 2           # fct streamed in 2 vocab halves
# gate column layout inside each r-block (32 cols) of the [128, 128] psum
# tile: i, f, o contiguous (one merged tanh), g last
GCOL = {0: 0, 1: 8, 2: 24, 3: 16}   # gate idx (i,f,g,o) -> col offset
HTW = (S + 1) * NB                  # per-k column width of the merged HT

_CACHE = {}
TRACE = False  # set by test.py to capture an NTFF profile


def _build_nc():
    if "nc" in _CACHE:
        return _CACHE["nc"]

    nc = bacc.Bacc(
        "TRN2",
        target_bir_lowering=False,
        debug=False,
        enable_asserts=False,
        num_devices=NCORES,
    )

    def din(name, shape, dt=BF16):
        return nc.dram_tensor(name, shape, dt, kind="ExternalInput").ap()

    # DMA issue order == declaration order below matters: enc_proj deps
    # (wenct, ift) first, then wct, then loop weights, fct last.
    ift_d = din("ift", [NE, 128, NB * PPAD])        # IF.T  [e, (b,j,q)] padded
    wenct_d = din("wenct", [NE, 128, ATT])          # wenc.T [e, a]
    wct_d = din("wct", [NE, 128, D4])               # Wc.T  [e, d4]
    whht_d = din("whht", [ND, 128, D4])             # (W_hh/2).T [dec, d4]
    wdect_d = din("wdect", [ND, 128, ATT])          # (wdec/2).T [dec, a]
    vt_d = din("vt", [NA, 128, 1])                  # V_w.T
    ept_d = din("ept", [128, S * NG * NB])          # embprojT [r, (t, m, b)]
    i128_d = din("i128", [128, 128])                # identity bf16
    fct_d = din("fct", [ND, 128, VOCAB])            # (fc/2).T [dec, vocab]
    h0t_d = din("h0t", [ND, 128, NB])               # 2*h0, bf16
    c0t_d = din("c0t", [ND, 128, NB], F32)          # 2*c0
    encb_d = din("encb", [NA, 128, 1], F32)         # wenc_b + wdec_b
    out_d = nc.dram_tensor("out", [S * NB, VOCAB], BF16,
                           kind="ExternalOutput").ap()

    with tile.TileContext(nc) as tc:
        from contextlib import ExitStack

        with ExitStack() as glob_ctx:
            gp = glob_ctx.enter_context(tc.tile_pool(name="glob", bufs=1))
            # persists from pre-loop into the loop
            ifw = [gp.tile([128, D4], BF16, name=f"ifw{i}", tag=f"ifw{i}")
                   for i in range(NBJ)]
            encp = [gp.tile([128, NB * PPAD], BF16, name=f"encp{i}",
                            tag=f"encp{i}") for i in range(NA)]
            # merged hidden state: [128, (k, t, b)] so the cell update is a
            # handful of strided instructions instead of 4x per-r chains
            HT = gp.tile([128, ND * HTW], BF16, name="HT", tag="HT")

            # ---------------- pre-loop: enc_proj and IFW ----------------
            with tc.tile_pool(name="pre", bufs=1) as pre:
                ift = [pre.tile([128, NB * PPAD], BF16, name=f"ift{k}",
                                tag=f"ift{k}") for k in range(NE)]
                wenct = [pre.tile([128, ATT], BF16, name=f"wen{k}",
                                  tag=f"wen{k}") for k in range(NE)]
                encb = [pre.tile([128, 1], F32, name=f"encb{i}",
                                 tag=f"encb{i}") for i in range(NA)]
                for k in range(NE):
                    nc.sync.dma_start(out=wenct[k], in_=wenct_d[k])
                    nc.sync.dma_start(out=ift[k], in_=ift_d[k])
                for i in range(NA):
                    nc.sync.dma_start(out=encb[i], in_=encb_d[i])

                with tc.tile_pool(name="pspre", bufs=4, space="PSUM") as pspre:
                    # enc_projT [a, (b,j,q)] += wenc_b + wdec_b
                    for i in range(NA):
                        for c in range(NB * PPAD // 512):
                            ps = pspre.tile([128, 512], F32, name="eps",
                                            tag="mm")
                            for k in range(NE):
                                nc.tensor.matmul(
                                    ps, wenct[k][:, ts(i, 128)],
                                    ift[k][:, ts(c, 512)],
                                    start=(k == 0), stop=(k == NE - 1))
                            dst = encp[i][:, ts(c, 512)]
                            if c % 2 == 0:
                                nc.scalar.activation(dst, ps, AF.Identity,
                                                     bias=encb[i])
                            else:
                                nc.vector.tensor_scalar_add(dst, ps, encb[i])

                # IFW = IF @ Wc.T, in two d4 halves to bound SBUF
                for half in range(2):
                    with tc.tile_pool(name=f"wc{half}", bufs=1) as wcp, \
                         tc.tile_pool(name=f"psw{half}", bufs=4,
                                      space="PSUM") as psw:
                        wch = [wcp.tile([128, 1024], BF16, name=f"wc{k}",
                                        tag=f"wc{k}") for k in range(NE)]
                        for k in range(NE):
                            nc.sync.dma_start(
                                out=wch[k],
                                in_=wct_d[k][:, half * 1024:(half + 1) * 1024])
                        for bj in range(NBJ):
                            for c in range(2):
                                ps = psw.tile([128, 512], F32, name="wps",
                                              tag="mm")
                                for k in range(NE):
                                    nc.tensor.matmul(
                                        ps, ift[k][:, ts(bj, 128)],
                                        wch[k][:, ts(c, 512)],
                                        start=(k == 0), stop=(k == NE - 1))
                                dst = ifw[bj][:, half * 1024 + c * 512:
                                              half * 1024 + (c + 1) * 512]
                                if (bj + c) % 2 == 0:
                                    nc.scalar.copy(out=dst, in_=ps)
                                else:
                                    nc.vector.tensor_copy(out=dst, in_=ps)

            # ---------------- recurrence ----------------
            with tc.tile_pool(name="lp", bufs=1) as lp, \
                 tc.tile_pool(name="psl", bufs=1, space="PSUM") as psl:
                att = [lp.tile([128, NB * PPAD], BF16, name=f"att{i}",
                               tag=f"att{i}") for i in range(NA)]
                whht = [lp.tile([128, D4], BF16, name=f"whht{k}",
                                tag=f"whht{k}") for k in range(ND)]
                wdect = [lp.tile([128, ATT], BF16, name=f"wdect{k}",
                                 tag=f"wdect{k}") for k in range(ND)]
                vt = [lp.tile([128, 1], BF16, name=f"vt{i}", tag=f"vt{i}")
                      for i in range(NA)]
                ept = lp.tile([128, S * NG * NB], BF16, name="ept")
                i128 = lp.tile([128, 128], BF16, name="i128")
                cT = lp.tile([128, ND * NB], F32, name="cT", tag="cT")
                hp_sb = [lp.tile([128, NB], F32, name=f"hp{i}", tag=f"hp{i}")
                         for i in range(NA)]
                rsum_sb = lp.tile([1, HB], F32, name="rsum", tag="rs", bufs=2)
                ones_col = lp.tile([128, 1], F32, name="ones_col")
                ones_row = lp.tile([1, 128], F32, name="ones_row")

                nc.sync.dma_start(out=i128, in_=i128_d)
                for k in range(ND):
                    nc.sync.dma_start(out=whht[k], in_=whht_d[k])
                    nc.sync.dma_start(out=wdect[k], in_=wdect_d[k])
                for i in range(NA):
                    nc.sync.dma_start(out=vt[i], in_=vt_d[i])
                nc.sync.dma_start(out=ept, in_=ept_d)
                for k in range(ND):
                    nc.sync.dma_start(out=HT[:, k * HTW:k * HTW + NB],
                                      in_=h0t_d[k])
                    nc.sync.dma_start(out=cT[:, k * NB:(k + 1) * NB],
                                      in_=c0t_d[k])
                # fct lives alongside the loop tiles so its DMA overlaps the
                # recurrence instead of waiting for the loop pool to free
                fcts = [lp.tile([128, VOCAB], BF16, name=f"fct{k}",
                                tag=f"fct{k}") for k in range(ND)]
                for k in range(ND):
                    nc.sync.dma_start(out=fcts[k], in_=fct_d[k])
                nc.vector.memset(ones_col, 1.0)
                nc.vector.memset(ones_row, 1.0)
                # zero the padded att columns once: tanh rewrites them with
                # tanh(0)=0 every step, so they stay zero; e/softmax pads
                # are excluded from sums and hit zero rows of IFW.
                for i in range(NA):
                    for b in range(NB):
                        nc.vector.memset(
                            att[i][:, b * PPAD + P:(b + 1) * PPAD], 0.0)

                for t in range(S):
                    hof = t * NB
                    hsl = [HT[:, k * HTW + hof:k * HTW + hof + NB]
                           for k in range(ND)]
                    # hprojT = (wdec/2) @ H~  (-> wdec @ h)
                    for i in range(NA):
                        ps = psl.tile([128, NB], F32, name="hps", tag="pa",
                                      bufs=2)
                        for k in range(ND):
                            nc.tensor.matmul(ps, wdect[k][:, ts(i, 128)],
                                             hsl[k], start=(k == 0),
                                             stop=(k == ND - 1))
                        nc.vector.tensor_copy(out=hp_sb[i], in_=ps)
                    # gates partial: W_hh@h + embproj (PE fills the tanh
                    # shadow; psum chains stay open for the ctx matmuls).
                    # All 16 column-blocks live in ONE [128, 128] psum tile.
                    # NOTE: matmul start=True marks the whole 2KB psum zero
                    # region pending-zero, so exactly ONE start per g tile
                    # per step; later first-writes of other column blocks
                    # self-initialize via the pending-zero bytes.
                    g_ps = psl.tile([128, 128], F32, name="g", tag="g",
                                    bufs=2)
                    for r in range(ND):
                        for gate in range(4):
                            m = gate * 4 + r
                            col = r * 32 + GCOL[gate]
                            for k in range(ND):
                                nc.tensor.matmul(
                                    g_ps[:, col:col + NB],
                                    whht[k][:, ts(m, 128)],
                                    hsl[k],
                                    start=(r == 0 and gate == 0 and k == 0),
                                    stop=False, skip_group_check=True)
                            ec = (t * NG + m) * NB
                            nc.tensor.matmul(g_ps[:, col:col + NB], i128,
                                             ept[:, ec:ec + NB],
                                             start=False, stop=False,
                                             skip_group_check=True)

                    for H in range(2):
                        b0 = H * HB
                        # att_in = encp + hp[b]; split vector/gpsimd so the
                        # adds don't serialize ahead of the tanh chain
                        for i in range(NA):
                            for bb in range(HB):
                                b = b0 + bb
                                lo = b * PPAD
                                nc.vector.tensor_scalar_add(
                                    att[i][:, lo:lo + P],
                                    encp[i][:, lo:lo + P],
                                    hp_sb[i][:, b:b + 1])
                        # tanh over the half, pads skipped via strided AP
                        # (pads were memset 0 once and never rewritten)
                        for i in range(NA):
                            sl = att[i][:, :].rearrange(
                                "p (b q) -> p b q", q=PPAD)[
                                :, b0:b0 + HB, 0:P]
                            nc.scalar.activation(sl, sl, AF.Tanh)
                        # e_T[q, bb] = V . att  (transposed, per half)
                        etp = [psl.tile([128, HB], F32, name="etp", tag="pa",
                                        bufs=2) for _ in range(NJ)]
                        for bb in range(HB):
                            lo = (b0 + bb) * PPAD
                            for j in range(NJ):
                                for i in range(NA):
                                    nc.tensor.matmul(
                                        etp[j][:, bb:bb + 1],
                                        att[i][:, lo + j * 128:
                                               lo + (j + 1) * 128],
                                        vt[i], start=(i == 0),
                                        stop=(i == NA - 1),
                                        skip_group_check=True)
                        # softmax without max-subtraction (|e| <= ~11)
                        ee = [lp.tile([128, HB], F32, name="ee", tag="ee",
                                      bufs=2) for _ in range(NJ)]
                        for j in range(NJ):
                            nc.scalar.activation(ee[j], etp[j], AF.Exp)
                        sum_ps = psl.tile([1, HB], F32, name="sum_ps",
                                          tag="pa", bufs=2)
                        nc.tensor.matmul(sum_ps, ones_col, ee[0],
                                         start=True, stop=False,
                                         skip_group_check=True)
                        nc.tensor.matmul(sum_ps, ones_col[0:P - 128],
                                         ee[1][0:P - 128, :],
                                         start=False, stop=True,
                                         skip_group_check=True)
                        nc.vector.reciprocal(rsum_sb, sum_ps)
                        rs_ps = psl.tile([128, HB], F32, name="rs_ps",
                                         tag="pa", bufs=2)
                        nc.tensor.matmul(rs_ps, ones_row, rsum_sb,
                                         start=True, stop=True)
                        wt = [lp.tile([128, HB], BF16, name="wt", tag="wt",
                                      bufs=2) for _ in range(NJ)]
                        for j in range(NJ):
                            nc.vector.tensor_mul(wt[j], ee[j], rs_ps)
                        # gates += attention context
                        for r in range(ND):
                            for gate in range(4):
                                m = gate * 4 + r
                                col = r * 32 + GCOL[gate]
                                for bb in range(HB):
                                    b = b0 + bb
                                    for j in range(NJ):
                                        last = (H == 1 and r == ND - 1
                                                and gate == 3
                                                and bb == HB - 1
                                                and j == NJ - 1)
                                        nc.tensor.matmul(
                                            g_ps[:, col + b:col + b + 1],
                                            ifw[b * NJ + j][:, ts(m, 128)],
                                            wt[j][:, bb:bb + 1],
                                            start=False, stop=last,
                                            skip_group_check=True)

                    # LSTM cell batched over all 4 r-tiles via strided APs:
                    # u = tanh(x/2) for i,f,o; C~ = .5(1+uf)C~ + (1+ui)tg;
                    # H~ = (1+uo)tanh(C~/2)
                    gv = g_ps[:, :].rearrange("p (r c) -> p r c", c=32)
                    u = lp.tile([128, ND * 24], F32, name="u", tag="u",
                                bufs=2)
                    uv = u[:, :].rearrange("p (r c) -> p r c", c=24)
                    tg = lp.tile([128, ND * NB], F32, name="tg", tag="tg",
                                 bufs=2)
                    tgv = tg[:, :].rearrange("p (r c) -> p r c", c=NB)
                    nc.scalar.activation(uv, gv[:, :, 0:24], AF.Tanh,
                                         scale=0.5)
                    nc.scalar.activation(tgv, gv[:, :, 24:32], AF.Tanh)
                    v1 = lp.tile([128, ND * NB], F32, name="v1", tag="v1",
                                 bufs=2)
                    v2 = lp.tile([128, ND * NB], F32, name="v2", tag="v2",
                                 bufs=2)
                    nc.vector.scalar_tensor_tensor(
                        v1[:, :].rearrange("p (r c) -> p r c", c=NB),
                        uv[:, :, 8:16], 1.0,
                        cT[:, :].rearrange("p (r c) -> p r c", c=NB),
                        ALU.add, ALU.mult)
                    nc.vector.scalar_tensor_tensor(
                        v2[:, :].rearrange("p (r c) -> p r c", c=NB),
                        uv[:, :, 0:8], 1.0, tgv, ALU.add, ALU.mult)
                    nc.vector.scalar_tensor_tensor(
                        cT, v1, 0.5, v2, ALU.mult, ALU.add)
                    th = lp.tile([128, ND * NB], F32, name="th", tag="th",
                                 bufs=2)
                    nc.scalar.activation(th, cT, AF.Tanh, scale=0.5)
                    nc.vector.scalar_tensor_tensor(
                        HT[:, :].rearrange("p (k c) -> p k c", c=HTW)[
                            :, :, hof + NB:hof + 2 * NB],
                        uv[:, :, 16:24], 1.0,
                        th[:, :].rearrange("p (r c) -> p r c", c=NB),
                        ALU.add, ALU.mult)

                # ---------------- tail: logits ----------------
                for m0, msz in ((0, 128), (128, S * NB - 128)):
                    for c in range(NVC):
                        ps = psl.tile([128, VC], F32, name="lps", tag="g",
                                      bufs=2)
                        for k in range(ND):
                            nc.tensor.matmul(
                                ps[:msz],
                                HT[:, k * HTW + NB + m0:
                                   k * HTW + NB + m0 + msz],
                                fcts[k][:, ts(c, VC)],
                                start=(k == 0), stop=(k == ND - 1))
                        lg = lp.tile([128, VC], BF16, name="lg",
                                     tag="lg", bufs=4)
                        if c % 2 == 0:
                            nc.scalar.copy(out=lg[:msz], in_=ps[:msz])
                        else:
                            nc.vector.tensor_copy(out=lg[:msz],
                                                  in_=ps[:msz])
                        nc.sync.dma_start(
                            out=out_d[m0:m0 + msz, ts(c, VC)],
                            in_=lg[:msz])

    nc.compile()
    _CACHE["nc"] = nc
    _CACHE["dbg"] = dict(ifw=ifw, encp=encp, HT=HT, att=att, hp_sb=hp_sb,
                         cT=cT, ept=ept)
    return nc


def _prep_core_inputs(image_feat, embproj, h0, c0, wct, wenct, whht, wdect,
                      vt, i128, fct, encb, core):
    bs = slice(core * NB, (core + 1) * NB)
    ifp = np.zeros((NB, PPAD, ENC), np.float32)
    ifp[:, :P, :] = image_feat[bs]
    # [e, (b, j, q)]
    ift = np.ascontiguousarray(
        ifp.reshape(NB * PPAD, ENC).T).astype(BF).reshape(NE, 128, NB * PPAD)
    ep = embproj[bs]                                   # [8, 20, 2048]
    ept = np.ascontiguousarray(
        ep.transpose(2, 1, 0)                          # [2048, 20, 8]
        .reshape(NG, 128, S, NB)                       # [m, r, t, b]
        .transpose(1, 2, 0, 3)                         # [r, t, m, b]
        .reshape(128, S * NG * NB)).astype(BF)
    h0t = np.ascontiguousarray(
        (2.0 * h0[bs]).T).reshape(ND, 128, NB).astype(BF)
    c0t = np.ascontiguousarray(
        (2.0 * c0[bs]).T).reshape(ND, 128, NB).astype(np.float32)
    return dict(ift=ift, wct=wct, wenct=wenct, whht=whht, wdect=wdect, vt=vt,
                ept=ept, i128=i128, fct=fct, h0t=h0t, c0t=c0t, encb=encb)


def kernel(image_feat, captions_ids, wenc_w, wenc_b, wdec_w, wdec_b,
           V_w, V_b, embed_w, h0_w, h0_b, c0_w, c0_b,
           W_ih, b_ih, W_hh, b_hh, fc_w, fc_b):
    image_feat = np.asarray(image_feat, np.float32)
    ids = np.asarray(captions_ids).astype(np.int64)

    # host-side glue (cheap, not on the device critical path)
    emb_seq = np.asarray(embed_w, np.float32)[ids]            # [B, S, EMB]
    We = np.asarray(W_ih, np.float32)[:, ENC:]                # [D4, EMB]
    Wc = np.asarray(W_ih, np.float32)[:, :ENC]                # [D4, ENC]
    embproj = emb_seq @ We.T + (np.asarray(b_ih) + np.asarray(b_hh))
    avg = image_feat.mean(axis=1)
    h0 = np.maximum(avg @ np.asarray(h0_w, np.float32).T + h0_b, 0.0)
    c0 = np.maximum(avg @ np.asarray(c0_w, np.float32).T + c0_b, 0.0)

    wct = np.ascontiguousarray(Wc.T).astype(BF).reshape(NE, 128, D4)
    wenct = np.ascontiguousarray(
        np.asarray(wenc_w, np.float32).T).astype(BF).reshape(NE, 128, ATT)
    # halved recurrence weights: device state is H~ = 2h
    whht = np.ascontiguousarray(
        0.5 * np.asarray(W_hh, np.float32).T).astype(BF).reshape(ND, 128, D4)
    wdect = np.ascontiguousarray(
        0.5 * np.asarray(wdec_w, np.float32).T).astype(BF).reshape(
            ND, 128, ATT)
    vtt = np.ascontiguousarray(
        np.asarray(V_w, np.float32)[0]).astype(BF).reshape(NA, 128, 1)
    i128 = np.eye(128, dtype=BF)
    fct = np.ascontiguousarray(
        0.5 * np.asarray(fc_w, np.float32).T).astype(BF).reshape(
            ND, 128, VOCAB)
    encb = (np.asarray(wenc_b, np.float32)
            + np.asarray(wdec_b, np.float32)).reshape(NA, 128, 1)

    nc = _build_nc()
    in_maps = [
        _prep_core_inputs(image_feat, embproj, h0, c0, wct, wenct, whht,
                          wdect, vtt, i128, fct, encb, c)
        for c in range(NCORES)
    ]
    res = run_bass_kernel_spmd(nc, in_maps, core_ids=list(range(NCORES)),
                               trace=TRACE)
    if TRACE:
        _CACHE["last_results"] = res

    preds = np.empty((B, S, VOCAB), np.float32)
    for c in range(NCORES):
        lg = res.results[c]["out"].astype(np.float32).reshape(S, NB, VOCAB)
        preds[c * NB:(c + 1) * NB] = lg.transpose(1, 0, 2)
    preds += np.asarray(fc_b, np.float32)
    return preds


if __name__ == "__main__":
    sys.path.insert(0, os.path.dirname(os.path.abspath(__file__)))
    import reference

    inputs = reference.setup_inputs()
    inputs = {k: np.asarray(v) for k, v in inputs.items()}
    expected = np.asarray(reference.reference(**inputs))
    actual = kernel(**inputs)
    err = np.abs(actual - expected)
    rel = np.linalg.norm(actual - expected) / np.linalg.norm(expected)
    print("max abs err:", err.max(), "rel:", rel)


# revision 37
# speedup vs baseline: 1.2072x; 1.2072x over previous
"""LSTM decoder with attention (image captioning) — Trainium2 Bass kernel.

Sharding: data-parallel over batch (64 images -> 8 cores x 8 images).
The whole per-step recurrence is collective-free; host does cheap glue
(embedding gather, weight transposes, h0/c0 init, final bias add).

Device program per core (b = 8 local images):
  pre:   enc_projT[a, (b,j,q)] = wenc @ IF.T + (wenc_b + wdec_b)
         IFW[(b,j,q), d4]      = IF @ Wc.T
  loop (t = 0..19, serial, pipelined over two b-halves):
         hprojT = wdec@h (PSUM) -> hp ; gates_T partial = W_hh@h + embproj
         per half: att = tanh(encp + hp[b]) ; e = V.att (transposed)
                   softmax (exp table) ; gates_T += sum_p w[b,p]*IFW[b,p,:]
         LSTM cell in tanh-only form: sigmoid(x) = (1+tanh(x/2))/2 with
         state rescaled (H~=2h, C~=2c, wdec/W_hh/fc pre-halved) so the
         scalar engine never loads the sigmoid table (stays on exp table).
  tail:  logits = H~.T @ (fc/2).T over all 20 steps, bf16 out.

All recurrence matmuls bf16; accumulation fp32.
"""

import os
import sys
import numpy as np

for _p in ("/opt/trn_rl_repo",):
    if _p not in sys.path and os.path.isdir(_p):
        sys.path.insert(0, _p)

import ml_dtypes  # noqa: E402

import concourse.bass as bass  # noqa: E402
import concourse.tile as tile  # noqa: E402
from concourse import bacc, mybir  # noqa: E402
from concourse.bass import ts  # noqa: E402
from concourse.bass_utils import run_bass_kernel_spmd  # noqa: E402

AF = mybir.ActivationFunctionType
ALU = mybir.AluOpType
F32 = mybir.dt.float32
BF16 = mybir.dt.bfloat16
F8 = mybir.dt.float8e4
DR = mybir.MatmulPerfMode.DoubleRow
BF = ml_dtypes.bfloat16

FP8_CTX = False  # attention-context matmuls in fp8e4m3 DoubleRow: the
                 # output quantization on short weighted sums costs too
                 # much accuracy (rel err 0.022 > 0.02 budget)
FP8_PRE = True   # enc_proj + IFW precompute in fp8e4m3 DoubleRow: inputs
                 # quantized but k=2048 dot products average the error
                 # away (~0.1%) and outputs stay bf16
WSCALE = 64.0    # wenc/Wc pre-scale so fp8 values clear the subnormals

# problem shapes (hardcoded)
VOCAB, ENC, EMB, DEC, ATT = 10000, 2048, 512, 512, 512
B, P, S = 64, 196, 20
NCORES = 8
NB = B // NCORES          # 8 images per core
HB = NB // 2              # 4 images per half (pipelined halves)
PPAD = 256                # P padded to 2 k-tiles per image
NJ = PPAD // 128          # 2
NBJ = NB * NJ             # 16 (b,j) k-tiles
NE = ENC // 128           # 16
NEH = NE // 2             # 8 DoubleRow k-pair tiles over ENC
NA = ATT // 128           # 4
ND = DEC // 128           # 4
NG = (4 * DEC) // 128     # 16 gate tiles
D4 = 4 * DEC              # 2048
NVC = 20                  # vocab chunks
VC = VOCAB // NVC         # 500
VH = VOCAB // 2           # fct streamed in 2 vocab halves
# gate column layout inside each r-block (32 cols) of the [128, 128] psum
# tile: i, f, o contiguous (one merged tanh), g last
GCOL = {0: 0, 1: 8, 2: 24, 3: 16}   # gate idx (i,f,g,o) -> col offset
HTW = (S + 1) * NB                  # per-k column width of the merged HT

_CACHE = {}
TRACE = False  # set by test.py to capture an NTFF profile


def _build_nc():
    if "nc" in _CACHE:
        return _CACHE["nc"]

    nc = bacc.Bacc(
        "TRN2",
        target_bir_lowering=False,
        debug=False,
        enable_asserts=False,
        num_devices=NCORES,
    )

    def din(name, shape, dt=BF16):
        return nc.dram_tensor(name, shape, dt, kind="ExternalInput").ap()

    # DMA issue order == declaration order below matters: enc_proj deps
    # (wenct, ift) first, then wct, then loop weights, fct last.
    if FP8_PRE:
        # DoubleRow plane-major layouts: [kk, 128, (plane, col)]
        ift_d = din("ift", [NEH, 128, 2 * NB * PPAD], F8)
        wenct_d = din("wenct", [NEH, 128, 2 * ATT], F8)
        wct_d = din("wct", [NEH, 128, 2 * D4], F8)
    else:
        ift_d = din("ift", [NE, 128, NB * PPAD])    # IF.T  [e, (b,j,q)]
        wenct_d = din("wenct", [NE, 128, ATT])      # wenc.T [e, a]
        wct_d = din("wct", [NE, 128, D4])           # Wc.T  [e, d4]
    whht_d = din("whht", [ND, 128, D4])             # (W_hh/2).T [dec, d4]
    wdect_d = din("wdect", [ND, 128, ATT])          # (wdec/2).T [dec, a]
    vt_d = din("vt", [NA, 128, 1])                  # V_w.T
    ept_d = din("ept", [128, S * NG * NB])          # embprojT [r, (t, m, b)]
    i128_d = din("i128", [128, 128])                # identity bf16
    fct_d = din("fct", [ND, 128, VOCAB])            # (fc/2).T [dec, vocab]
    h0t_d = din("h0t", [ND, 128, NB])               # 2*h0, bf16
    c0t_d = din("c0t", [ND, 128, NB], F32)          # 2*c0
    encb_d = din("encb", [NA, 128, 1], F32)         # wenc_b + wdec_b
    out_d = nc.dram_tensor("out", [S * NB, VOCAB], BF16,
                           kind="ExternalOutput").ap()

    with tile.TileContext(nc) as tc:
        from contextlib import ExitStack

        with ExitStack() as glob_ctx:
            gp = glob_ctx.enter_context(tc.tile_pool(name="glob", bufs=1))
            # persists from pre-loop into the loop. With FP8_CTX the two
            # p-chunks (j) of each image live plane-major in one fp8 tile
            # consumed by DoubleRow matmuls.
            if FP8_CTX:
                ifw = [gp.tile([128, NJ * D4], F8, name=f"ifw{b}",
                               tag=f"ifw{b}") for b in range(NB)]
            else:
                ifw = [gp.tile([128, D4], BF16, name=f"ifw{i}",
                               tag=f"ifw{i}") for i in range(NBJ)]
            encp = [gp.tile([128, NB * PPAD], BF16, name=f"encp{i}",
                            tag=f"encp{i}") for i in range(NA)]
            # merged hidden state: [128, (k, t, b)] so the cell update is a
            # handful of strided instructions instead of 4x per-r chains
            HT = gp.tile([128, ND * HTW], BF16, name="HT", tag="HT")

            # ---------------- pre-loop: enc_proj and IFW ----------------
            NK = NEH if FP8_PRE else NE
            ESC = 1.0 / WSCALE if FP8_PRE else 1.0

            def pre_mm(ps, lhs_t, lhs_sl, rhs_t, rhs_sl, kk, nk):
                if FP8_PRE:
                    lhs = lhs_t[:, :].rearrange(
                        "p (two m) -> p two m", two=2)[:, :, lhs_sl]
                    rhs = rhs_t[:, :].rearrange(
                        "p (two m) -> p two m", two=2)[:, :, rhs_sl]
                    nc.tensor.matmul(ps, lhs, rhs, perf_mode=DR,
                                     start=(kk == 0), stop=(kk == nk - 1))
                else:
                    nc.tensor.matmul(ps, lhs_t[:, lhs_sl], rhs_t[:, rhs_sl],
                                     start=(kk == 0), stop=(kk == nk - 1))

            with tc.tile_pool(name="pre", bufs=1) as pre:
                pdt = F8 if FP8_PRE else BF16
                pw = 2 if FP8_PRE else 1
                ift = [pre.tile([128, pw * NB * PPAD], pdt, name=f"ift{k}",
                                tag=f"ift{k}") for k in range(NK)]
                wenct = [pre.tile([128, pw * ATT], pdt, name=f"wen{k}",
                                  tag=f"wen{k}") for k in range(NK)]
                encb = [pre.tile([128, 1], F32, name=f"encb{i}",
                                 tag=f"encb{i}") for i in range(NA)]
                for k in range(NK):
                    nc.sync.dma_start(out=wenct[k], in_=wenct_d[k])
                    nc.sync.dma_start(out=ift[k], in_=ift_d[k])
                for i in range(NA):
                    nc.sync.dma_start(out=encb[i], in_=encb_d[i])

                with tc.tile_pool(name="pspre", bufs=4, space="PSUM") as pspre:
                    # enc_projT [a, (b,j,q)] += wenc_b + wdec_b
                    for i in range(NA):
                        for c in range(NB * PPAD // 512):
                            ps = pspre.tile([128, 512], F32, name="eps",
                                            tag="mm")
                            for kk in range(NK):
                                pre_mm(ps, wenct[kk], ts(i, 128),
                                       ift[kk], ts(c, 512), kk, NK)
                            dst = encp[i][:, ts(c, 512)]
                            if c % 2 == 0:
                                nc.scalar.activation(dst, ps, AF.Identity,
                                                     bias=encb[i], scale=ESC)
                            else:
                                nc.vector.tensor_scalar(
                                    dst, ps, ESC, encb[i],
                                    ALU.mult, ALU.add)

                # IFW = IF @ Wc.T, in two d4 halves to bound SBUF
                for half in range(2):
                    with tc.tile_pool(name=f"wc{half}", bufs=1) as wcp, \
                         tc.tile_pool(name=f"psw{half}", bufs=4,
                                      space="PSUM") as psw:
                        wch = [wcp.tile([128, pw * 1024], pdt,
                                        name=f"wc{k}", tag=f"wc{k}")
                               for k in range(NK)]
                        for k in range(NK):
                            if FP8_PRE:
                                nc.sync.dma_start(
                                    out=wch[k],
                                    in_=wct_d[k].rearrange(
                                        "p (two m) -> p two m", two=2)[
                                        :, :, half * 1024:(half + 1) * 1024])
                            else:
                                nc.sync.dma_start(
                                    out=wch[k],
                                    in_=wct_d[k][:, half * 1024:
                                                 (half + 1) * 1024])
                        for bj in range(NBJ):
                            for c in range(2):
                                ps = psw.tile([128, 512], F32, name="wps",
                                              tag="mm")
                                for kk in range(NK):
                                    pre_mm(ps, ift[kk], ts(bj, 128),
                                           wch[kk], ts(c, 512), kk, NK)
                                co = half * 1024 + c * 512
                                if FP8_CTX:
                                    b, j = bj // NJ, bj % NJ
                                    dst = ifw[b][:, j * D4 + co:
                                                 j * D4 + co + 512]
                                else:
                                    dst = ifw[bj][:, co:co + 512]
                                if (bj + c) % 2 == 0:
                                    nc.scalar.activation(dst, ps, AF.Copy,
                                                         scale=ESC)
                                else:
                                    nc.vector.tensor_scalar_mul(dst, ps, ESC)

            # ---------------- recurrence ----------------
            with tc.tile_pool(name="lp", bufs=1) as lp, \
                 tc.tile_pool(name="psl", bufs=1, space="PSUM") as psl:
                att = [lp.tile([128, NB * PPAD], BF16, name=f"att{i}",
                               tag=f"att{i}") for i in range(NA)]
                whht = [lp.tile([128, D4], BF16, name=f"whht{k}",
                                tag=f"whht{k}") for k in range(ND)]
                wdect = [lp.tile([128, ATT], BF16, name=f"wdect{k}",
                                 tag=f"wdect{k}") for k in range(ND)]
                vt = [lp.tile([128, 1], BF16, name=f"vt{i}", tag=f"vt{i}")
                      for i in range(NA)]
                ept = lp.tile([128, S * NG * NB], BF16, name="ept")
                i128 = lp.tile([128, 128], BF16, name="i128")
                cT = lp.tile([128, ND * NB], F32, name="cT", tag="cT")
                hp_sb = [lp.tile([128, NB], F32, name=f"hp{i}", tag=f"hp{i}")
                         for i in range(NA)]
                rsum_sb = lp.tile([1, HB], F32, name="rsum", tag="rs", bufs=2)
                ones_col = lp.tile([128, 1], F32, name="ones_col")
                ones_row = lp.tile([1, 128], F32, name="ones_row")

                nc.sync.dma_start(out=i128, in_=i128_d)
                for k in range(ND):
                    nc.sync.dma_start(out=whht[k], in_=whht_d[k])
                    nc.sync.dma_start(out=wdect[k], in_=wdect_d[k])
                for i in range(NA):
                    nc.sync.dma_start(out=vt[i], in_=vt_d[i])
                nc.sync.dma_start(out=ept, in_=ept_d)
                for k in range(ND):
                    nc.sync.dma_start(out=HT[:, k * HTW:k * HTW + NB],
                                      in_=h0t_d[k])
                    nc.sync.dma_start(out=cT[:, k * NB:(k + 1) * NB],
                                      in_=c0t_d[k])
                # fct lives alongside the loop tiles so its DMA overlaps the
                # recurrence instead of waiting for the loop pool to free
                fcts = [lp.tile([128, VOCAB], BF16, name=f"fct{k}",
                                tag=f"fct{k}") for k in range(ND)]
                for k in range(ND):
                    nc.sync.dma_start(out=fcts[k], in_=fct_d[k])
                nc.vector.memset(ones_col, 1.0)
                nc.vector.memset(ones_row, 1.0)
                # zero the padded att columns once: tanh rewrites them with
                # tanh(0)=0 every step, so they stay zero; e/softmax pads
                # are excluded from sums and hit zero rows of IFW.
                for i in range(NA):
                    for b in range(NB):
                        nc.vector.memset(
                            att[i][:, b * PPAD + P:(b + 1) * PPAD], 0.0)

                for t in range(S):
                    hof = t * NB
                    hsl = [HT[:, k * HTW + hof:k * HTW + hof + NB]
                           for k in range(ND)]
                    # hprojT = (wdec/2) @ H~  (-> wdec @ h)
                    for i in range(NA):
                        ps = psl.tile([128, NB], F32, name="hps", tag="pa",
                                      bufs=2)
                        for k in range(ND):
                            nc.tensor.matmul(ps, wdect[k][:, ts(i, 128)],
                                             hsl[k], start=(k == 0),
                                             stop=(k == ND - 1))
                        nc.vector.tensor_copy(out=hp_sb[i], in_=ps)
                    # gates partial: W_hh@h + embproj (PE fills the tanh
                    # shadow; psum chains stay open for the ctx matmuls).
                    # All 16 column-blocks live in ONE [128, 128] psum tile.
                    # NOTE: matmul start=True marks the whole 2KB psum zero
                    # region pending-zero, so exactly ONE start per g tile
                    # per step; later first-writes of other column blocks
                    # self-initialize via the pending-zero bytes.
                    g_ps = psl.tile([128, 128], F32, name="g", tag="g",
                                    bufs=2)
                    for r in range(ND):
                        for gate in range(4):
                            m = gate * 4 + r
                            col = r * 32 + GCOL[gate]
                            for k in range(ND):
                                nc.tensor.matmul(
                                    g_ps[:, col:col + NB],
                                    whht[k][:, ts(m, 128)],
                                    hsl[k],
                                    start=(r == 0 and gate == 0 and k == 0),
                                    stop=False, skip_group_check=True)
                            ec = (t * NG + m) * NB
                            nc.tensor.matmul(g_ps[:, col:col + NB], i128,
                                             ept[:, ec:ec + NB],
                                             start=False, stop=False,
                                             skip_group_check=True)

                    def att_block(H):
                        # att_in = encp + hp[b] (vector); tanh per i-tile
                        # over the half, pads skipped via strided AP
                        b0 = H * HB
                        for i in range(NA):
                            for bb in range(HB):
                                b = b0 + bb
                                lo = b * PPAD
                                nc.vector.tensor_scalar_add(
                                    att[i][:, lo:lo + P],
                                    encp[i][:, lo:lo + P],
                                    hp_sb[i][:, b:b + 1])
                        for i in range(NA):
                            sl = att[i][:, :].rearrange(
                                "p (b q) -> p b q", q=PPAD)[
                                :, b0:b0 + HB, 0:P]
                            nc.scalar.activation(sl, sl, AF.Tanh)

                    def e_sm_block(H):
                        # e_T[q, bb] = V . att; softmax without
                        # max-subtraction (|e| <= ~11); returns wt
                        b0 = H * HB
                        etp = [psl.tile([128, HB], F32, name="etp", tag="pa",
                                        bufs=2) for _ in range(NJ)]
                        for bb in range(HB):
                            lo = (b0 + bb) * PPAD
                            for j in range(NJ):
                                for i in range(NA):
                                    nc.tensor.matmul(
                                        etp[j][:, bb:bb + 1],
                                        att[i][:, lo + j * 128:
                                               lo + (j + 1) * 128],
                                        vt[i], start=(i == 0),
                                        stop=(i == NA - 1),
                                        skip_group_check=True)
                        ee = [lp.tile([128, HB], F32, name="ee", tag="ee",
                                      bufs=2) for _ in range(NJ)]
                        for j in range(NJ):
                            nc.scalar.activation(ee[j], etp[j], AF.Exp)
                        sum_ps = psl.tile([1, HB], F32, name="sum_ps",
                                          tag="pa", bufs=2)
                        nc.tensor.matmul(sum_ps, ones_col, ee[0],
                                         start=True, stop=False,
                                         skip_group_check=True)
                        nc.tensor.matmul(sum_ps, ones_col[0:P - 128],
                                         ee[1][0:P - 128, :],
                                         start=False, stop=True,
                                         skip_group_check=True)
                        nc.vector.reciprocal(rsum_sb, sum_ps)
                        rs_ps = psl.tile([128, HB], F32, name="rs_ps",
                                         tag="pa", bufs=2)
                        nc.tensor.matmul(rs_ps, ones_row, rsum_sb,
                                         start=True, stop=True)
                        if FP8_CTX:
                            wt = lp.tile([128, NJ * HB], F8, name="wt",
                                         tag="wt", bufs=4)
                            for j in range(NJ):
                                nc.vector.tensor_mul(
                                    wt[:, j * HB:(j + 1) * HB], ee[j], rs_ps)
                        else:
                            wt = [lp.tile([128, HB], BF16, name="wt",
                                          tag="wt", bufs=4)
                                  for _ in range(NJ)]
                            for j in range(NJ):
                                nc.vector.tensor_mul(wt[j], ee[j], rs_ps)
                        return wt

                    def ctx_block(H, wt, rset):
                        b0 = H * HB
                        for r in rset:
                            for gate in range(4):
                                m = gate * 4 + r
                                col = r * 32 + GCOL[gate]
                                for bb in range(HB):
                                    b = b0 + bb
                                    last = (H == 1 and r == ND - 1
                                            and gate == 3 and bb == HB - 1)
                                    if FP8_CTX:
                                        # one DoubleRow matmul covers both
                                        # p-chunks (k=256)
                                        lhs = ifw[b][:, :].rearrange(
                                            "p (two m) -> p two m",
                                            two=NJ)[:, :, ts(m, 128)]
                                        rhs = wt[:, :].rearrange(
                                            "p (two c) -> p two c",
                                            two=NJ)[:, :, bb:bb + 1]
                                        nc.tensor.matmul(
                                            g_ps[:, col + b:col + b + 1],
                                            lhs, rhs, perf_mode=DR,
                                            start=False, stop=last,
                                            skip_group_check=True)
                                    else:
                                        for j in range(NJ):
                                            nc.tensor.matmul(
                                                g_ps[:, col + b:col + b + 1],
                                                ifw[b * NJ + j][:, ts(m, 128)],
                                                wt[j][:, bb:bb + 1],
                                                start=False,
                                                stop=last and j == NJ - 1,
                                                skip_group_check=True)

                    # interleave so the PE never sits behind a softmax
                    # chain: eB runs between the two ctxA halves
                    att_block(0)
                    wtA = e_sm_block(0)
                    att_block(1)
                    ctx_block(0, wtA, (0, 1))
                    wtB = e_sm_block(1)
                    ctx_block(0, wtA, (2, 3))
                    ctx_block(1, wtB, (0, 1, 2, 3))

                    # LSTM cell batched over all 4 r-tiles via strided APs:
                    # u = tanh(x/2) for i,f,o; C~ = .5(1+uf)C~ + (1+ui)tg;
                    # H~ = (1+uo)tanh(C~/2)
                    gv = g_ps[:, :].rearrange("p (r c) -> p r c", c=32)
                    u = lp.tile([128, ND * 24], F32, name="u", tag="u",
                                bufs=2)
                    uv = u[:, :].rearrange("p (r c) -> p r c", c=24)
                    tg = lp.tile([128, ND * NB], F32, name="tg", tag="tg",
                                 bufs=2)
                    tgv = tg[:, :].rearrange("p (r c) -> p r c", c=NB)
                    nc.scalar.activation(uv, gv[:, :, 0:24], AF.Tanh,
                                         scale=0.5)
                    nc.scalar.activation(tgv, gv[:, :, 24:32], AF.Tanh)
                    v1 = lp.tile([128, ND * NB], F32, name="v1", tag="v1",
                                 bufs=2)
                    v2 = lp.tile([128, ND * NB], F32, name="v2", tag="v2",
                                 bufs=2)
                    nc.vector.scalar_tensor_tensor(
                        v1[:, :].rearrange("p (r c) -> p r c", c=NB),
                        uv[:, :, 8:16], 1.0,
                        cT[:, :].rearrange("p (r c) -> p r c", c=NB),
                        ALU.add, ALU.mult)
                    nc.vector.scalar_tensor_tensor(
                        v2[:, :].rearrange("p (r c) -> p r c", c=NB),
                        uv[:, :, 0:8], 1.0, tgv, ALU.add, ALU.mult)
                    nc.vector.scalar_tensor_tensor(
                        cT, v1, 0.5, v2, ALU.mult, ALU.add)
                    th = lp.tile([128, ND * NB], F32, name="th", tag="th",
                                 bufs=2)
                    nc.scalar.activation(th, cT, AF.Tanh, scale=0.5)
                    nc.vector.scalar_tensor_tensor(
                        HT[:, :].rearrange("p (k c) -> p k c", c=HTW)[
                            :, :, hof + NB:hof + 2 * NB],
                        uv[:, :, 16:24], 1.0,
                        th[:, :].rearrange("p (r c) -> p r c", c=NB),
                        ALU.add, ALU.mult)

                # ---------------- tail: logits ----------------
                for m0, msz in ((0, 128), (128, S * NB - 128)):
                    for c in range(NVC):
                        ps = psl.tile([128, VC], F32, name="lps", tag="g",
                                      bufs=2)
                        for k in range(ND):
                            nc.tensor.matmul(
                                ps[:msz],
                                HT[:, k * HTW + NB + m0:
                                   k * HTW + NB + m0 + msz],
                                fcts[k][:, ts(c, VC)],
                                start=(k == 0), stop=(k == ND - 1))
                        lg = lp.tile([128, VC], BF16, name="lg",
                                     tag="lg", bufs=4)
                        if c % 2 == 0:
                            nc.scalar.copy(out=lg[:msz], in_=ps[:msz])
                        else:
                            nc.vector.tensor_copy(out=lg[:msz],
                                                  in_=ps[:msz])
                        nc.sync.dma_start(
                            out=out_d[m0:m0 + msz, ts(c, VC)],
                            in_=lg[:msz])

    nc.compile()
    _CACHE["nc"] = nc
    _CACHE["dbg"] = dict(ifw=ifw, encp=encp, HT=HT, att=att, hp_sb=hp_sb,
                         cT=cT, ept=ept)
    return nc


def _drpack(x, scale=1.0):
    """[NE, 128, cols] fp32 -> DoubleRow plane-major [NEH, 128, 2*cols] fp8."""
    c = x.shape[2]
    return np.ascontiguousarray(
        (x * scale).reshape(NEH, 2, 128, c).transpose(0, 2, 1, 3)
    ).reshape(NEH, 128, 2 * c).astype(ml_dtypes.float8_e4m3)


def _prep_core_inputs(image_feat, embproj, h0, c0, wct, wenct, whht, wdect,
                      vt, i128, fct, encb, core):
    bs = slice(core * NB, (core + 1) * NB)
    ifp = np.zeros((NB, PPAD, ENC), np.float32)
    ifp[:, :P, :] = image_feat[bs]
    # [e, (b, j, q)]
    ift32 = np.ascontiguousarray(
        ifp.reshape(NB * PPAD, ENC).T).reshape(NE, 128, NB * PPAD)
    if FP8_PRE:
        ift = _drpack(ift32)
    else:
        ift = ift32.astype(BF)
    ep = embproj[bs]                                   # [8, 20, 2048]
    ept = np.ascontiguousarray(
        ep.transpose(2, 1, 0)                          # [2048, 20, 8]
        .reshape(NG, 128, S, NB)                       # [m, r, t, b]
        .transpose(1, 2, 0, 3)                         # [r, t, m, b]
        .reshape(128, S * NG * NB)).astype(BF)
    h0t = np.ascontiguousarray(
        (2.0 * h0[bs]).T).reshape(ND, 128, NB).astype(BF)
    c0t = np.ascontiguousarray(
        (2.0 * c0[bs]).T).reshape(ND, 128, NB).astype(np.float32)
    return dict(ift=ift, wct=wct, wenct=wenct, whht=whht, wdect=wdect, vt=vt,
                ept=ept, i128=i128, fct=fct, h0t=h0t, c0t=c0t, encb=encb)


def kernel(image_feat, captions_ids, wenc_w, wenc_b, wdec_w, wdec_b,
           V_w, V_b, embed_w, h0_w, h0_b, c0_w, c0_b,
           W_ih, b_ih, W_hh, b_hh, fc_w, fc_b):
    image_feat = np.asarray(image_feat, np.float32)
    ids = np.asarray(captions_ids).astype(np.int64)

    # host-side glue (cheap, not on the device critical path)
    emb_seq = np.asarray(embed_w, np.float32)[ids]            # [B, S, EMB]
    We = np.asarray(W_ih, np.float32)[:, ENC:]                # [D4, EMB]
    Wc = np.asarray(W_ih, np.float32)[:, :ENC]                # [D4, ENC]
    embproj = emb_seq @ We.T + (np.asarray(b_ih) + np.asarray(b_hh))
    avg = image_feat.mean(axis=1)
    h0 = np.maximum(avg @ np.asarray(h0_w, np.float32).T + h0_b, 0.0)
    c0 = np.maximum(avg @ np.asarray(c0_w, np.float32).T + c0_b, 0.0)

    wct32 = np.ascontiguousarray(Wc.T).reshape(NE, 128, D4)
    wenct32 = np.ascontiguousarray(
        np.asarray(wenc_w, np.float32).T).reshape(NE, 128, ATT)
    if FP8_PRE:
        wct = _drpack(wct32, WSCALE)
        wenct = _drpack(wenct32, WSCALE)
    else:
        wct = wct32.astype(BF)
        wenct = wenct32.astype(BF)
    # halved recurrence weights: device state is H~ = 2h
    whht = np.ascontiguousarray(
        0.5 * np.asarray(W_hh, np.float32).T).astype(BF).reshape(ND, 128, D4)
    wdect = np.ascontiguousarray(
        0.5 * np.asarray(wdec_w, np.float32).T).astype(BF).reshape(
            ND, 128, ATT)
    vtt = np.ascontiguousarray(
        np.asarray(V_w, np.float32)[0]).astype(BF).reshape(NA, 128, 1)
    i128 = np.eye(128, dtype=BF)
    fct = np.ascontiguousarray(
        0.5 * np.asarray(fc_w, np.float32).T).astype(BF).reshape(
            ND, 128, VOCAB)
    encb = (np.asarray(wenc_b, np.float32)
            + np.asarray(wdec_b, np.float32)).reshape(NA, 128, 1)

    nc = _build_nc()
    in_maps = [
        _prep_core_inputs(image_feat, embproj, h0, c0, wct, wenct, whht,
                          wdect, vtt, i128, fct, encb, c)
        for c in range(NCORES)
    ]
    res = run_bass_kernel_spmd(nc, in_maps, core_ids=list(range(NCORES)),
                               trace=TRACE)
    if TRACE:
        _CACHE["last_results"] = res

    preds = np.empty((B, S, VOCAB), np.float32)
    for c in range(NCORES):
        lg = res.results[c]["out"].astype(np.float32).reshape(S, NB, VOCAB)
        preds[c * NB:(c + 1) * NB] = lg.transpose(1, 0, 2)
    preds += np.asarray(fc_b, np.float32)
    return preds


if __name__ == "__main__":
    sys.path.insert(0, os.path.dirname(os.path.abspath(__file__)))
    import reference

    inputs = reference.setup_inputs()
    inputs = {k: np.asarray(v) for k, v in inputs.items()}
    expected = np.asarray(reference.reference(**inputs))
    actual = kernel(**inputs)
    err = np.abs(actual - expected)
    rel = np.linalg.norm(actual - expected) / np.linalg.norm(expected)
    print("max abs err:", err.max(), "rel:", rel)
